# revision 1
# baseline (speedup 1.0000x reference)
"""DSSM (vision Mamba block) Trainium2 kernel.

Problem: B=4, H=W=48, L=2304, D_MODEL=96, D_INNER=192, N=16, R=6, K=3.

Sharding (8 cores, no device-to-device comms):
  core c -> batch b=c//2, d_inner half=c%2. Each core runs the full-d
  front-end (in_proj, depthwise conv, x_dbl) for its batch, the K=3
  selective scans for its 96 d_inner channels, and a partial out_proj
  (contraction over its d-half). Host sums the two partials per batch.

Channel permutation trick: all per-core weights are permuted host-side so
the core's own d-half always occupies channels 0..95 -> one SPMD program.

Scan: p=(d8,n16) partition layout (n-minor), hardware tensor_tensor_scan
along t. Decay a = exp(A_n * delta) via a tiny PE matmul (contraction 8)
plus one big ACT exp pass. Scan-path tensors are bf16 (its contribution to
the output is ~1e-5 relative; validated numerically); main path is fp32.
"""

import numpy as np
import ml_dtypes

import concourse.bass as bass
import concourse.mybir as mybir
import concourse.tile as tile
from concourse.bass_utils import run_bass_kernel_spmd

# ---------------------------------------------------------------- tile fix
# The walrus here accepts only ONE inline sem-wait per instruction; Tile can
# attach several. Hoist extras onto same-engine NOPs placed just before.
_wsplit_counter = [0]


def _split_multi_waits(nc):
    for fn in nc.m.functions:
        for blk in fn.blocks:
            out = []
            changed = False
            for inst in blk.instructions:
                si = inst.sync_info
                waits = list(si.on_wait) if si is not None and si.on_wait else []
                if len(waits) > 1:
                    changed = True
                    for w in waits[:-1]:
                        _wsplit_counter[0] += 1
                        nop = mybir.InstNoOp(name=f"wsplit-{_wsplit_counter[0]}")
                        nop.engine = inst.engine
                        nop.sync_info = mybir.SyncInfo(on_wait=[w], on_update=[])
                        out.append(nop)
                    inst.sync_info = mybir.SyncInfo(
                        on_wait=[waits[-1]],
                        on_update=list(si.on_update) if si.on_update else [],
                    )
                out.append(inst)
            if changed:
                blk.instructions = out


class TileContextFixed(tile.TileContext):
    def __exit__(self, exc_type, exc_val, exc_tb):
        r = super().__exit__(exc_type, exc_val, exc_tb)
        if exc_type is None:
            _split_multi_waits(self.nc)
        return r


# ---------------------------------------------------------------- constants
B, H, W = 4, 48, 48
DM, DI, N, R, K = 96, 192, 16, 6, 3
L = H * W
DH = 96          # d-half per core
G = DH // 8      # 12 groups of 8 channels
TILES = [(0, 480), (480, 960), (960, 1440), (1440, 1920), (1920, 2304)]
TILES9 = [(0, 960), (960, 1920), (1920, 2304)]

F32 = mybir.dt.float32
BF16 = mybir.dt.bfloat16
MUL = mybir.AluOpType.mult
ADD = mybir.AluOpType.add
AF = mybir.ActivationFunctionType

_COMPILED = {}


def _build_nc():
    nc = bass.Bass()

    # ---- dram I/O (per-core values supplied via in_maps)
    x_nat = nc.dram_tensor("x_nat", [L, DM], F32, kind="ExternalInput")
    wxz_T = nc.dram_tensor("wxz_T", [DM, 288], F32, kind="ExternalInput")
    conv_diag = nc.dram_tensor("conv_diag", [DH, 18 * DH], F32, kind="ExternalInput")
    conv_bias = nc.dram_tensor("conv_bias", [DH, 2], F32, kind="ExternalInput")
    xp_T = nc.dram_tensor("xp_T", [DH, K * 2 * 80], F32, kind="ExternalInput")
    dtw_T = nc.dram_tensor("dtw_T", [R, K * DH], BF16, kind="ExternalInput")
    dt_bias = nc.dram_tensor("dt_bias", [DH, K], F32, kind="ExternalInput")
    wa8 = nc.dram_tensor("wa8", [DH, K * G * 128], BF16, kind="ExternalInput")
    wr = nc.dram_tensor("wr", [128, G * DH], BF16, kind="ExternalInput")
    wbc = nc.dram_tensor("wbc", [80, 128], BF16, kind="ExternalInput")
    wi8 = nc.dram_tensor("wi8", [DH, G * 128], BF16, kind="ExternalInput")
    ident = nc.dram_tensor("ident", [128, 128], F32, kind="ExternalInput")
    ds_t = nc.dram_tensor("ds_t", [DH, K], F32, kind="ExternalInput")
    wout_T = nc.dram_tensor("wout_T", [DH, DM], F32, kind="ExternalInput")
    out_part = nc.dram_tensor("out_part", [DM, L], F32, kind="ExternalOutput")

    with TileContextFixed(nc) as tc:
        with (
            tc.tile_pool(name="wts", bufs=1) as wts,
            tc.tile_pool(name="big", bufs=1) as big,
            tc.tile_pool(name="perk", bufs=2) as perk,
            tc.tile_pool(name="stream", bufs=4) as stream,
            tc.tile_pool(name="hpool", bufs=3) as hpool,
            tc.tile_pool(name="psF", bufs=2, space="PSUM") as psF,
            tc.tile_pool(name="psA", bufs=2, space="PSUM") as psA,
            tc.tile_pool(name="psD", bufs=2, space="PSUM") as psD,
            tc.tile_pool(name="psY", bufs=2, space="PSUM") as psY,
        ):
            # ---- load weights
            def wload(dram, shape, dtype):
                t = wts.tile(shape, dtype, tag=dram.name + "_s", name=dram.name + "_s")
                nc.sync.dma_start(t[:, :], dram[:, :])
                return t

            s_wxz = wload(wxz_T, [DM, 288], F32)
            s_cd = wload(conv_diag, [DH, 18 * DH], F32)
            s_cb = wload(conv_bias, [DH, 2], F32)
            s_xp = wload(xp_T, [DH, K * 2 * 80], F32)
            s_dtw = wload(dtw_T, [R, K * DH], BF16)
            s_dtb = wload(dt_bias, [DH, K], F32)
            s_wa8 = wload(wa8, [DH, K * G * 128], BF16)
            s_wr = wload(wr, [128, G * DH], BF16)
            s_wbc = wload(wbc, [80, 128], BF16)
            s_wi8 = wload(wi8, [DH, G * 128], BF16)
            s_id = wload(ident, [128, 128], F32)
            s_ds = wload(ds_t, [DH, K], F32)
            s_wout = wload(wout_T, [DH, DM], F32)

            # ---- x load natural, transpose on PE: (L, 96) -> [96, L]
            xT = big.tile([DM, L], F32, tag="xT")
            for i in range(L // 128):
                xn = stream.tile([128, DM], F32, tag="xn")
                nc.sync.dma_start(xn[:, :], x_nat[128 * i : 128 * i + 128, :])
                pst_full = psF.tile([DM, 480], F32, tag="psF", name="pst_full")
                nc.tensor.transpose(pst_full[:, 0:128], xn[:, :], s_id[:, :])
                nc.scalar.copy(xT[:, 128 * i : 128 * i + 128], pst_full[:, 0:128])

            # ---- pads for conv (one per half), zeroed borders
            pads = [big.tile([DH, 50 * 50], F32, tag=f"pad{h}", name=f"pad{h}") for h in range(2)]
            for p in pads:
                nc.vector.memset(p[:, :], 0.0)

            # ---- in_proj: xc (both halves, into pad layout) + z half
            zs = big.tile([DH, L], F32, tag="zs")
            for it, (t0, t1) in enumerate(TILES):
                tw = t1 - t0
                rows = tw // 48
                for hh in range(2):
                    ps = psF.tile([DH, 480], F32, tag="psF")
                    nc.tensor.matmul(
                        ps[:, :tw], s_wxz[:, 96 * hh : 96 * hh + 96], xT[:, t0:t1]
                    )
                    dst = pads[hh][:, :].rearrange("p (r c) -> p r c", r=50, c=50)[
                        :, 1 + 10 * it : 1 + 10 * it + rows, 1:49
                    ]
                    src = ps[:, :tw].rearrange("p (r c) -> p r c", r=rows, c=48)
                    nc.scalar.copy(dst, src)
                ps = psF.tile([DH, 480], F32, tag="psF")
                nc.tensor.matmul(ps[:, :tw], s_wxz[:, 192:288], xT[:, t0:t1])
                nc.scalar.activation(zs[:, t0:t1], ps[:, :tw], AF.Silu)

            # ---- depthwise conv 3x3 + bias + silu -> u (per half)
            us = [big.tile([DH, L], F32, tag=f"u{h}", name=f"u{h}") for h in range(2)]
            for hh in range(2):
                for rb in range(6):
                    ps = psF.tile([DH, 480], F32, tag="psF")
                    j = 0
                    for dy in range(3):
                        for dx in range(3):
                            src = pads[hh][:, :].rearrange(
                                "p (r c) -> p r c", r=50, c=50
                            )[:, 8 * rb + dy : 8 * rb + dy + 8, dx : dx + 48]
                            nc.tensor.matmul(
                                ps[:, :384],
                                s_cd[:, (hh * 9 + j) * DH : (hh * 9 + j + 1) * DH],
                                src,
                                start=(j == 0),
                                stop=(j == 8),
                            )
                            j += 1
                    nc.scalar.activation(
                        us[hh][:, rb * 384 : rb * 384 + 384],
                        ps[:, :384],
                        AF.Silu,
                        bias=s_cb[:, hh : hh + 1],
                    )

            # ---- x_dbl per direction: [80, L] bf16, sections dt@0 B@32 C@64
            xdbls = [big.tile([80, L], BF16, tag=f"xdbl{k}", name=f"xdbl{k}")
                     for k in range(K)]
            for t0, t1 in TILES:
                tw = t1 - t0
                for k in range(K):
                    ps = psF.tile([80, 480], F32, tag="psF")
                    nc.tensor.matmul(
                        ps[:, :tw], s_xp[:, (2 * k) * 80 : (2 * k + 1) * 80],
                        us[0][:, t0:t1], start=True, stop=False,
                    )
                    nc.tensor.matmul(
                        ps[:, :tw], s_xp[:, (2 * k + 1) * 80 : (2 * k + 2) * 80],
                        us[1][:, t0:t1], start=False, stop=True,
                    )
                    nc.scalar.copy(xdbls[k][:, t0:t1], ps[:, :tw])

            # ---- per-direction scan
            # ys buffers reuse space freed by front-end tensors (pads, xT)
            ysb = [
                big.tile([DH, L], F32, tag="pad0", name="ysb0"),
                big.tile([DH, L], F32, tag="pad1", name="ysb1"),
                big.tile([DH, L], F32, tag="xT", name="ysb2"),
            ]
            for k in range(K):
                # xk: permuted copy of xdbl_k (sections dt@0, B@32, C@64)
                if k == 0:
                    xk = xdbls[0]
                elif k == 1:
                    xk = perk.tile([80, L], BF16, tag="xk")
                    src = xdbls[1][:, :].rearrange("p (h w) -> p w h", h=H, w=W)
                    nc.scalar.copy(
                        xk[:, :].rearrange("p (a b) -> p a b", a=W, b=H), src
                    )
                else:
                    xk = perk.tile([80, L], BF16, tag="xk")
                    nc.scalar.copy(xk[:, :], xdbls[2][:, ::-1])

                # u_k: permuted u (own half = channels 0..95 -> us[0])
                if k == 0:
                    u_k = us[0]
                elif k == 1:
                    u_k = perk.tile([DH, L], F32, tag="uk")
                    src = us[0][:, :].rearrange("p (h w) -> p w h", h=H, w=W)
                    nc.scalar.copy(u_k[:, :].rearrange("p (a b) -> p a b", a=W, b=H), src)
                else:
                    u_k = perk.tile([DH, L], F32, tag="uk")
                    nc.scalar.copy(u_k[:, :], us[0][:, ::-1])

                # B/C partition-broadcasts (n-minor): [128, L] bf16 via PE
                b_b = perk.tile([128, L], BF16, tag="b_b")
                c_b = perk.tile([128, L], BF16, tag="c_b")
                for t0, t1 in TILES:
                    tw = t1 - t0
                    psb = psA.tile([128, 480], F32, tag="psA")
                    nc.tensor.matmul(psb[:, :tw], s_wbc[32:48, :], xk[32:48, t0:t1])
                    nc.scalar.copy(b_b[:, t0:t1], psb[:, :tw])
                    psc = psA.tile([128, 480], F32, tag="psA")
                    nc.tensor.matmul(psc[:, :tw], s_wbc[64:80, :], xk[64:80, t0:t1])
                    nc.scalar.copy(c_b[:, t0:t1], psc[:, :tw])

                delta = perk.tile([DH, L], BF16, tag="delta")
                du = perk.tile([DH, L], BF16, tag="du")
                carry = perk.tile([128, G], BF16, tag="carry")
                for it, (t0, t1) in enumerate(TILES9):
                    tw = t1 - t0
                    halves = [(t0 + 480 * q, min(t0 + 480 * (q + 1), t1))
                              for q in range((tw + 479) // 480)]
                    # delta = ln(exp(v) + 1), v = dtw @ dts + bias
                    for q0, q1 in halves:
                        psv = psF.tile([DH, 480], F32, tag="psF")
                        nc.tensor.matmul(
                            psv[:, : q1 - q0],
                            s_dtw[:, k * DH : (k + 1) * DH],
                            xk[0:R, q0:q1],
                        )
                        ev = stream.tile([DH, 480], F32, tag="ev")
                        nc.scalar.activation(
                            ev[:, : q1 - q0], psv[:, : q1 - q0], AF.Exp,
                            bias=s_dtb[:, k : k + 1],
                        )
                        nc.scalar.activation(
                            delta[:, q0:q1], ev[:, : q1 - q0], AF.Ln, bias=1.0
                        )
                    nc.vector.tensor_mul(du[:, t0:t1], delta[:, t0:t1], u_k[:, t0:t1])

                    psy_h = []
                    for g in range(G):
                        a_t = stream.tile([128, 960], BF16, tag="a")
                        dub = stream.tile([128, 960], BF16, tag="dub")
                        for qi, (q0, q1) in enumerate(halves):
                            qw = q1 - q0
                            psa = psA.tile([128, 480], F32, tag="psA")
                            nc.tensor.matmul(
                                psa[:, :qw],
                                s_wa8[:, (k * G + g) * 128 : (k * G + g + 1) * 128],
                                delta[:, q0:q1],
                            )
                            nc.scalar.activation(
                                a_t[:, q0 - t0 : q1 - t0], psa[:, :qw], AF.Exp
                            )
                            psd = psD.tile([128, 480], F32, tag="psD")
                            nc.tensor.matmul(
                                psd[:, :qw],
                                s_wi8[:, g * 128 : (g + 1) * 128],
                                du[:, q0:q1],
                            )
                            nc.scalar.copy(dub[:, q0 - t0 : q1 - t0], psd[:, :qw])
                        w_t = stream.tile([128, 960], BF16, tag="w")
                        nc.vector.tensor_mul(w_t[:, :tw], dub[:, :tw], b_b[:, t0:t1])
                        h_t = hpool.tile([128, 960], BF16, tag="h")
                        init = 0.0 if it == 0 else carry[:, g : g + 1]
                        nc.vector.tensor_tensor_scan(
                            h_t[:, :tw], a_t[:, :tw], w_t[:, :tw], init, MUL, ADD
                        )
                        if it < len(TILES9) - 1:
                            nc.vector.tensor_copy(
                                carry[:, g : g + 1], h_t[:, tw - 1 : tw]
                            )
                        ch = stream.tile([128, 960], BF16, tag="ch")
                        nc.vector.tensor_mul(ch[:, :tw], h_t[:, :tw], c_b[:, t0:t1])
                        if g == 0:
                            psy_h = [psY.tile([DH, 480], F32, tag="psY", name=f"psy{qi2}")
                                     for qi2 in range(len(halves))]
                        for qi, (q0, q1) in enumerate(halves):
                            nc.tensor.matmul(
                                psy_h[qi][:, : q1 - q0],
                                s_wr[:, g * DH : (g + 1) * DH],
                                ch[:, q0 - t0 : q1 - t0],
                                start=(g == 0),
                                stop=(g == G - 1),
                            )
                    # ys = u_k * Ds + y
                    for qi, (q0, q1) in enumerate(halves):
                        nc.vector.scalar_tensor_tensor(
                            ysb[k][:, q0:q1],
                            u_k[:, q0:q1],
                            s_ds[:, k : k + 1],
                            psy_h[qi][:, : q1 - q0],
                            MUL,
                            ADD,
                        )

            # ---- merge directions (undo orderings, in-place in ysb0), gate
            p1 = ysb[1][:, :].rearrange("p (w h) -> p h w", w=W, h=H)
            nc.vector.tensor_add(
                ysb[0][:, :].rearrange("p (a b) -> p a b", a=H, b=W),
                ysb[0][:, :].rearrange("p (a b) -> p a b", a=H, b=W),
                p1,
            )
            nc.vector.tensor_add(ysb[0][:, :], ysb[0][:, :], ysb[2][:, ::-1])
            yg = ysb[0]
            nc.vector.tensor_mul(yg[:, :], yg[:, :], zs[:, :])

            out_sb = big.tile([DM, L], F32, tag="zs", name="out_sb")
            for t0, t1 in TILES:
                tw = t1 - t0
                ps = psF.tile([DM, 480], F32, tag="psF")
                nc.tensor.matmul(ps[:, :tw], s_wout[:, :], yg[:, t0:t1])
                nc.scalar.copy(out_sb[:, t0:t1], ps[:, :tw])
                nc.sync.dma_start(out_part[:, t0:t1], out_sb[:, t0:t1])

    return nc


def _prep_in_maps(inputs):
    f32 = lambda a: np.ascontiguousarray(np.asarray(a, np.float32))
    bf16 = lambda a: np.ascontiguousarray(
        np.asarray(a, np.float32).astype(ml_dtypes.bfloat16)
    )
    x = f32(inputs["x"])
    in_proj_w = f32(inputs["in_proj_w"])        # (384, 96)
    conv_w = f32(inputs["conv_w"]).reshape(DI, 9)
    conv_b = f32(inputs["conv_b"])
    x_proj_w = f32(inputs["x_proj_w"])          # (K, 38, 192)
    dt_w = f32(inputs["dt_projs_w"])            # (K, 192, 6)
    dt_b = f32(inputs["dt_projs_b"])            # (K, 192)
    A = -np.exp(f32(inputs["A_logs"])).reshape(K, DI, N)
    Ds = f32(inputs["Ds"]).reshape(K, DI)
    out_w = f32(inputs["out_proj_w"])           # (96, 192)

    wr_np = np.zeros((128, G * DH), np.float32)
    for g in range(G):
        for d8 in range(8):
            wr_np[d8 * 16 : d8 * 16 + 16, g * DH + g * 8 + d8] = 1.0

    in_maps = []
    for c in range(8):
        b, half = c // 2, c % 2
        pd = np.concatenate([np.arange(DI)[96 * half : 96 * half + 96],
                             np.arange(DI)[96 * (1 - half) : 96 * (1 - half) + 96]])
        dh = pd[:DH]

        wxz = np.zeros((DM, 288), np.float32)
        wxz[:, 0:96] = in_proj_w[pd[:96]].T
        wxz[:, 96:192] = in_proj_w[pd[96:]].T
        wxz[:, 192:288] = in_proj_w[DI + dh].T

        cd = np.zeros((DH, 18 * DH), np.float32)
        for hh in range(2):
            ch_idx = pd[hh * 96 : hh * 96 + 96]
            for j in range(9):
                blk = np.zeros((DH, DH), np.float32)
                np.fill_diagonal(blk, conv_w[ch_idx, j])
                cd[:, (hh * 9 + j) * DH : (hh * 9 + j + 1) * DH] = blk
        cb = np.stack([conv_b[pd[:96]], conv_b[pd[96:]]], axis=1)

        xp = np.zeros((DH, K * 2 * 80), np.float32)
        for k in range(K):
            for hh in range(2):
                blk = np.zeros((DH, 80), np.float32)
                ch_idx = pd[hh * 96 : hh * 96 + 96]
                blk[:, 0:6] = x_proj_w[k][0:6, ch_idx].T
                blk[:, 32:48] = x_proj_w[k][6:22, ch_idx].T
                blk[:, 64:80] = x_proj_w[k][22:38, ch_idx].T
                xp[:, (2 * k + hh) * 80 : (2 * k + hh + 1) * 80] = blk

        dtw = np.zeros((R, K * DH), np.float32)
        for k in range(K):
            dtw[:, k * DH : (k + 1) * DH] = dt_w[k][dh].T
        dtb = np.stack([dt_b[k][dh] for k in range(K)], axis=1)

        wa = np.zeros((DH, K * G * 128), np.float32)
        for k in range(K):
            for g in range(G):
                blk = np.zeros((DH, 128), np.float32)
                for d8 in range(8):
                    blk[g * 8 + d8, d8 * 16 : d8 * 16 + 16] = A[k, dh[g * 8 + d8]]
                wa[:, (k * G + g) * 128 : (k * G + g + 1) * 128] = blk

        ds_np = np.stack([Ds[k][dh] for k in range(K)], axis=1)

        wi8_np = np.zeros((DH, G * 128), np.float32)
        for g in range(G):
            for d8 in range(8):
                wi8_np[g * 8 + d8, g * 128 + d8 * 16 : g * 128 + d8 * 16 + 16] = 1.0

        wbc_np = np.zeros((80, 128), np.float32)
        for n in range(16):
            wbc_np[32 + n, n::16] = 1.0
            wbc_np[64 + n, n::16] = 1.0

        in_maps.append(
            dict(
                x_nat=x[b].reshape(L, DM),
                wxz_T=wxz,
                conv_diag=cd,
                conv_bias=np.ascontiguousarray(cb),
                xp_T=xp,
                dtw_T=dtw.astype(ml_dtypes.bfloat16),
                dt_bias=np.ascontiguousarray(dtb),
                wa8=wa.astype(ml_dtypes.bfloat16),
                wr=wr_np.astype(ml_dtypes.bfloat16),
                wbc=wbc_np.astype(ml_dtypes.bfloat16),
                wi8=wi8_np.astype(ml_dtypes.bfloat16),
                ident=np.eye(128, dtype=np.float32),
                ds_t=np.ascontiguousarray(ds_np),
                wout_T=np.ascontiguousarray(out_w[:, dh].T),
            )
        )
    return in_maps


def kernel(**inputs):
    if "nc" not in _COMPILED:
        _COMPILED["nc"] = _build_nc()
    nc = _COMPILED["nc"]
    in_maps = _prep_in_maps(inputs)
    res = run_bass_kernel_spmd(nc, in_maps, core_ids=list(range(8)))
    out = np.zeros((B, H, W, DM), np.float32)
    for b in range(B):
        p = res.results[2 * b]["out_part"] + res.results[2 * b + 1]["out_part"]
        out[b] = p.T.reshape(H, W, DM)
    return out

